# revision 75
# baseline (speedup 1.0000x reference)
"""Trainium2 Bass kernel for nn_Actor_47278999995139 (dense_mlp, memory-bound).

Reference computation (per row of x, B=262144 rows):
    4-layer MLP 64->32->24->16->8; each layer:
        y   = (h_in * 0.99) @ (w * g).T        (g = lognormal weight noise)
        q   = clip(round(y * 128) / 128, -1, 1)   (+ tiny thermal noise, omitted)
        h   = relu(q)   (layers 1-3);  out = tanh(q)  (layer 4)

Strategy: pure data parallel over 8 NeuronCores (32768 rows/core).
  * All scale factors fold into host-precomputed weight matrices, so the
    device works in the h*128 integer domain:
        A1 = (128*0.99*w1*g1).T      psum1 = x @ A1           = y1*128
        Ak = (0.99*wk*gk).T          psumk = h_{k-1}s @ Ak    = yk*128
    and h_s = clip(round(y*128), 0, 128) feeds the next matmul directly.
  * Host pre-transposes x so the device sees [features, batch] tiles
    (contraction on partitions, no on-device transposes). x is sent as an
    exact fp16 hi/lo split (same 4 B/elem as fp32): hi on partitions 0-63,
    lo on 64-127, so layer 1 is two K=128 fp16 matmuls with [A1;A1]-stacked
    split weights. Layers 2-4 use fp16 hi+lo split weights (exact to 2^-22);
    h values are integers 0..128, exact in fp16.
  * Batch is packed 4096-rows-per-superchunk across all 128 partitions as
    4 diagonal (row,col) tile-groups of 32 - the four groups' matmuls run
    concurrently in the PE array (wave-ordered: all hi then all lo), and
    DVE/ACT lanes stay full for the elementwise work.
  * clip+round exploits that the hardware fp32->int16 output convert
    rounds half-to-even, exactly matching jnp.round: layer 1 is one DVE
    tensor_scalar (max,min)->int16 + cast; layers 2/3 are one ACT
    Relu->int16 + one DVE min->fp16; layer 4 is one ACT Copy->int16.
    (The python CoreSim diverges here - it truncates - so numerical
    correctness is validated against the reference on hardware.)
  * The emission is software-pipelined 4 deep (stage l of superchunk i-l at
    iteration i) so TensorE never waits on the current superchunk's
    elementwise chain and stays HAM-warm.
  * Layer-4's +-1 clip is dropped: P(|y4|>1) is ~0 (y4 std ~0.16) and tanh
    saturates; output is written as fp16 (integers + tanh, exact enough).
  * Per-weight lognormal noise is reproduced exactly on host with jax CPU;
    per-element thermal noise (sigma=8e-7) is omitted (~5.4e-3 relative
    error, dominated by rounding-bin flips at quantization boundaries -
    reproducing it exactly would require streaming 84 MB of host-generated
    noise, doubling the memory traffic).
"""

import os
import sys

import numpy as np

sys.path.insert(0, "/opt/trn_rl_repo")

B = 262144
STATE_DIM = 64
ACTION_DIM = 8
N_CORES = 8
ROWS_PER_CORE = B // N_CORES          # 32768
HALF = ROWS_PER_CORE // 2             # 16384 (batch cols per partition-half)
SC_ROWS = 4096                        # rows per (full) superchunk
N_SC = ROWS_PER_CORE // SC_ROWS       # 8
CFD = SC_ROWS // 4                    # psum free dim per batch group (1024)


def _schedule():
    """Superchunk (offset, rows) list."""
    return [(j * SC_ROWS, SC_ROWS) for j in range(N_SC)]
MAGIC = 12582912.0                    # 1.5 * 2**23
IR_DROP = 0.99
LOGNORMAL_SIGMA = 0.12

_CACHE = {}


def _noise_factors(w_shapes):
    """Reproduce the reference's per-weight lognormal conductance noise."""
    import jax

    cpu = jax.devices("cpu")[0]
    with jax.default_device(cpu):
        import jax.numpy as jnp

        ks = jax.random.split(jax.random.key(42), 8)
        gs = []
        for i, shape in enumerate(w_shapes):
            g = jnp.exp(
                jax.random.normal(ks[2 * i], shape, dtype=jnp.float32)
                * LOGNORMAL_SIGMA
            )
            gs.append(np.asarray(g, dtype=np.float32))
    return gs


def _build_nc(n_sc):
    from contextlib import ExitStack

    import concourse.tile as tile
    from concourse import bacc, mybir

    dt = mybir.dt
    AF = mybir.ActivationFunctionType
    AL = mybir.AluOpType

    nc = bacc.Bacc("TRN2", target_bir_lowering=False, debug=False)
    scs = _schedule() if n_sc == N_SC else [
        (j * SC_ROWS, SC_ROWS) for j in range(n_sc)
    ]
    rows = sum(r for _, r in scs)
    xT = nc.dram_tensor("xt", [128, rows], dt.float16, kind="ExternalInput")
    wA1 = nc.dram_tensor("wa1", [128, 64], dt.float16, kind="ExternalInput")
    wF = nc.dram_tensor("wf", [128, 192], dt.float16, kind="ExternalInput")
    out = nc.dram_tensor("out", [128, rows // 4], dt.float16, kind="ExternalOutput")

    with tile.TileContext(nc) as tc, ExitStack() as ctx:
        wpool = ctx.enter_context(tc.tile_pool(name="w", bufs=1))
        xpool = ctx.enter_context(tc.tile_pool(name="x", bufs=3))
        spool = ctx.enter_context(tc.tile_pool(name="s", bufs=4))
        opool = ctx.enter_context(tc.tile_pool(name="o", bufs=3))
        ppools = [
            ctx.enter_context(tc.tile_pool(name=f"p{i}", bufs=1, space="PSUM"))
            for i in range(1, 5)
        ]

        w1s = wpool.tile([128, 64], dt.float16, tag="w1s")
        nc.sync.dma_start(w1s[:], wA1[:, :])
        wfs = wpool.tile([128, 192], dt.float16, tag="wfs")
        nc.sync.dma_start(wfs[:], wF[:, :])

        def split_mm(ps, hi_cols, lo_cols, rhs, k, cfd):
            """fp16 hi+lo weight matmuls, wave-ordered (all hi, then all lo)
            so the four diagonal tile-groups can run concurrently."""
            for cols, st, sp in ((hi_cols, True, False), (lo_cols, False, True)):
                for ch in range(0, cfd, 512):
                    for g in range(4):
                        nc.tensor.matmul(
                            ps[32 * g : 32 * g + 32, ch : ch + 512],
                            wfs[32 * g : 32 * g + k, cols[0] : cols[1]],
                            rhs[32 * g : 32 * g + k, ch : ch + 512],
                            start=st,
                            stop=sp,
                            skip_group_check=True,
                            tile_position=(32 * g, 32 * g),
                        )

        # Software-pipelined emission: at iteration i, stage-l work runs on
        # superchunk i-l, so every TensorE instruction's inputs were produced
        # in an earlier iteration and PE never stalls on the current
        # superchunk's elementwise chain (also keeps PE dense -> HAM warm).
        s1s, s2s, s3s = {}, {}, {}
        xts = {}
        n = len(scs)

        def load_x(j):
            off, rws = scs[j]
            xt = xpool.tile([128, rws], dt.float16, tag="xt")
            nc.sync.dma_start(xt[:], xT[:, off : off + rws])
            xts[j] = xt

        # HAM warmup: ~3.4us of dependency-free PE work overlapping the first
        # x DMA, so the real matmuls start at the un-throttled clock.
        wu = spool.tile([128, 512], dt.float16, tag="wu")
        nc.vector.memset(wu[:], 1.0)
        pw = ppools[0].tile([128, 512], dt.float32, tag="ps1")
        for r in range(8):
            nc.tensor.matmul(
                pw[0:32, :],
                wu[0:128, 0:32],
                wu[0:128, :],
                start=(r == 0),
                stop=(r == 7),
                skip_group_check=True,
                tile_position=(0, 0),
            )

        load_x(0)
        for i in range(n + 3):
            if i < n:
                _, rws = scs[i]
                cfd = rws // 4
                xt = xts.pop(i)

                # layer 1: xt holds xh (partitions 0-63) and xl (64-127) for
                # the same batch. Wave A: K=128 with [A1h;A1h] stacked computes
                # (xh+xl)@A1h = x@A1h; wave B: [A1l;A1l] adds x@A1l.
                ps1 = ppools[0].tile([128, cfd], dt.float32, tag="ps1")
                for wc, st, sp in (((0, 32), True, False), ((32, 64), False, True)):
                    for ch in range(0, cfd, 512):
                        for g in range(4):
                            nc.tensor.matmul(
                                ps1[32 * g : 32 * g + 32, ch : ch + 512],
                                w1s[0:128, wc[0] : wc[1]],
                                xt[0:128, g * cfd + ch : g * cfd + ch + 512],
                                start=st,
                                stop=sp,
                                skip_group_check=True,
                                tile_position=(0, 32 * g),
                            )
                # clamp+round in one op: fp32->int16 convert rounds-to-nearest-even
                t1 = spool.tile([128, cfd], dt.int16, tag="t1")
                nc.vector.tensor_scalar(
                    t1[:], ps1[:], 0.0, 128.0, op0=AL.max, op1=AL.min
                )
                s1 = spool.tile([128, cfd], dt.float16, tag="s1")
                nc.vector.tensor_copy(s1[:], t1[:])
                s1s[i] = (s1, cfd)

            if 0 <= i - 1 < n:
                s1, cfd = s1s.pop(i - 1)
                ps2 = ppools[1].tile([128, cfd], dt.float32, tag="ps2")
                split_mm(ps2, (0, 32), (32, 64), s1, 32, cfd)
                # relu+round in one ACT op (int16 out converts with RNE);
                # then clamp-top+cast in one DVE op
                u2 = spool.tile([128, cfd], dt.int16, tag="u2")
                nc.scalar.activation(u2[:], ps2[:], AF.Relu)
                s2 = spool.tile([128, cfd], dt.float16, tag="s2")
                nc.vector.tensor_scalar(s2[:], u2[:], 128.0, None, op0=AL.min)
                s2s[i - 1] = (s2, cfd)

            if 0 <= i - 2 < n:
                s2, cfd = s2s.pop(i - 2)
                ps3 = ppools[2].tile([128, cfd], dt.float32, tag="ps3")
                split_mm(ps3, (64, 96), (96, 128), s2, 24, cfd)
                u3 = spool.tile([128, cfd], dt.int16, tag="u3")
                nc.scalar.activation(u3[:], ps3[:], AF.Relu)
                s3 = spool.tile([128, cfd], dt.float16, tag="s3")
                nc.vector.tensor_scalar(s3[:], u3[:], 128.0, None, op0=AL.min)
                s3s[i - 2] = (s3, cfd)

            if 0 <= i - 3 < n:
                off, rws = scs[i - 3]
                s3, cfd = s3s.pop(i - 3)
                ps4 = ppools[3].tile([128, cfd], dt.float32, tag="ps4")
                split_mm(ps4, (128, 160), (160, 192), s3, 16, cfd)
                q4 = spool.tile([128, cfd], dt.int16, tag="q4")
                nc.scalar.activation(q4[:], ps4[:], AF.Copy)
                o = opool.tile([128, cfd], dt.float16, tag="o")
                nc.scalar.activation(o[:], q4[:], AF.Tanh, scale=1.0 / 128.0)
                nc.sync.dma_start(out[:, off // 4 : off // 4 + cfd], o[:])

            if 0 <= i + 1 < n:
                load_x(i + 1)

    return nc


def _prep_weights(w1, w2, w3, w4):
    gs = _noise_factors([w.shape for w in (w1, w2, w3, w4)])
    A1 = (128.0 * IR_DROP * (w1.astype(np.float32) * gs[0])).T.astype(np.float32)
    A2 = (IR_DROP * (w2.astype(np.float32) * gs[1])).T.astype(np.float32)  # [32,24]
    A3 = (IR_DROP * (w3.astype(np.float32) * gs[2])).T.astype(np.float32)  # [24,16]
    A4 = (IR_DROP * (w4.astype(np.float32) * gs[3])).T.astype(np.float32)  # [16, 8]

    def split16(a):
        hi = a.astype(np.float16)
        lo = (a - hi.astype(np.float32)).astype(np.float16)
        return hi, lo

    A1h, A1l = split16(A1)
    w1pack = np.zeros((128, 64), dtype=np.float16)
    w1pack[:, 0:32] = np.vstack([A1h, A1h])
    w1pack[:, 32:64] = np.vstack([A1l, A1l])

    A2h, A2l = split16(A2)
    A3h, A3l = split16(A3)
    A4h, A4l = split16(A4)
    wfpack = np.zeros((128, 192), dtype=np.float16)
    for g in range(4):
        wfpack[32 * g : 32 * g + 32, 0:24] = A2h
        wfpack[32 * g : 32 * g + 32, 32:56] = A2l
        wfpack[32 * g : 32 * g + 24, 64:80] = A3h
        wfpack[32 * g : 32 * g + 24, 96:112] = A3l
        wfpack[32 * g : 32 * g + 16, 128:136] = A4h
        wfpack[32 * g : 32 * g + 16, 160:168] = A4l
    return w1pack, wfpack


def _pack_x_shard(xs):
    """[rows, 64] fp32 -> [128, rows] fp16: partitions 0-63 hold xh (fp16 hi),
    partitions 64-127 hold xl (fp16 residual), batch in natural order."""
    xs = xs.astype(np.float32)
    xh = xs.astype(np.float16)
    xl = (xs - xh.astype(np.float32)).astype(np.float16)
    return np.ascontiguousarray(np.concatenate([xh.T, xl.T], axis=0))


def _decode_out(dev_out, rows):
    """[128, rows//4] fp16 -> [rows, 8] fp32; per superchunk (off, rws) the
    block cols off//4.. holds feature r of batch off + g*cfd + c at row
    32g+r, col off//4 + c."""
    res = np.empty((rows, ACTION_DIM), dtype=np.float32)
    f32 = dev_out.astype(np.float32)
    for off, rws in _schedule():
        cfd = rws // 4
        blk = f32[:, off // 4 : off // 4 + cfd]
        for g in range(4):
            base = off + g * cfd
            res[base : base + cfd, :] = blk[32 * g : 32 * g + 8, :].T
    return res


def run(inputs, trace=False):
    x = np.asarray(inputs["x"], dtype=np.float32)
    assert x.shape == (B, STATE_DIM)
    w1pack, wfpack = _prep_weights(
        np.asarray(inputs["w1"]), np.asarray(inputs["w2"]),
        np.asarray(inputs["w3"]), np.asarray(inputs["w4"]),
    )

    if "nc" not in _CACHE:
        nc = _build_nc(N_SC)
        if not nc.is_finalized():
            nc.finalize()
        _CACHE["nc"] = nc
    nc = _CACHE["nc"]

    in_maps = []
    for c in range(N_CORES):
        shard = x[c * ROWS_PER_CORE : (c + 1) * ROWS_PER_CORE]
        in_maps.append(
            {"xt": _pack_x_shard(shard), "wa1": w1pack, "wf": wfpack}
        )

    from concourse.bass_utils import run_bass_kernel_spmd

    res = run_bass_kernel_spmd(
        nc, in_maps, core_ids=list(range(N_CORES)), trace=trace
    )

    full = np.empty((B, ACTION_DIM), dtype=np.float32)
    for c in range(N_CORES):
        full[c * ROWS_PER_CORE : (c + 1) * ROWS_PER_CORE] = _decode_out(
            res.results[c]["out"], ROWS_PER_CORE
        )
    return full, res.exec_time_ns


def kernel(**inputs):
    out, _ = run(inputs)
    return out


# revision 76
# speedup vs baseline: 1.0402x; 1.0402x over previous
"""Trainium2 Bass kernel for nn_Actor_47278999995139 (dense_mlp, memory-bound).

Reference computation (per row of x, B=262144 rows):
    4-layer MLP 64->32->24->16->8; each layer:
        y   = (h_in * 0.99) @ (w * g).T        (g = lognormal weight noise)
        q   = clip(round(y * 128) / 128, -1, 1)   (+ tiny thermal noise, omitted)
        h   = relu(q)   (layers 1-3);  out = tanh(q)  (layer 4)

Strategy: pure data parallel over 8 NeuronCores (32768 rows/core).
  * All scale factors fold into host-precomputed weight matrices, so the
    device works in the h*128 integer domain:
        A1 = (128*0.99*w1*g1).T      psum1 = x @ A1           = y1*128
        Ak = (0.99*wk*gk).T          psumk = h_{k-1}s @ Ak    = yk*128
    and h_s = clip(round(y*128), 0, 128) feeds the next matmul directly.
  * Host pre-transposes x so the device sees [features, batch] tiles
    (contraction on partitions, no on-device transposes). x is sent as an
    exact fp16 hi/lo split (same 4 B/elem as fp32): hi on partitions 0-63,
    lo on 64-127, so layer 1 is two K=128 fp16 matmuls with [A1;A1]-stacked
    split weights. Layers 2-4 use fp16 hi+lo split weights (exact to 2^-22);
    h values are integers 0..128, exact in fp16.
  * Batch is packed 4096-rows-per-superchunk across all 128 partitions as
    4 diagonal (row,col) tile-groups of 32 - the four groups' matmuls run
    concurrently in the PE array (wave-ordered: all hi then all lo), and
    DVE/ACT lanes stay full for the elementwise work.
  * clip+round exploits that the hardware fp32->int16 output convert
    rounds half-to-even, exactly matching jnp.round: layer 1 is one DVE
    tensor_scalar (max,min)->int16 + cast; layers 2/3 are one ACT
    Relu->int16 + one DVE min->fp16; layer 4 is one ACT Copy->int16.
    (The python CoreSim diverges here - it truncates - so numerical
    correctness is validated against the reference on hardware.)
  * The emission is software-pipelined 4 deep (stage l of superchunk i-l at
    iteration i) so TensorE never waits on the current superchunk's
    elementwise chain and stays HAM-warm.
  * Layer-4's +-1 clip is dropped: P(|y4|>1) is ~0 (y4 std ~0.16) and tanh
    saturates; output is written as fp16 (integers + tanh, exact enough).
  * Per-weight lognormal noise is reproduced exactly on host with jax CPU;
    per-element thermal noise (sigma=8e-7) is omitted (~5.4e-3 relative
    error, dominated by rounding-bin flips at quantization boundaries -
    reproducing it exactly would require streaming 84 MB of host-generated
    noise, doubling the memory traffic).
"""

import os
import sys

import numpy as np

sys.path.insert(0, "/opt/trn_rl_repo")

B = 262144
STATE_DIM = 64
ACTION_DIM = 8
N_CORES = 8
ROWS_PER_CORE = B // N_CORES          # 32768
HALF = ROWS_PER_CORE // 2             # 16384 (batch cols per partition-half)
SC_ROWS = 4096                        # rows per (full) superchunk
N_SC = ROWS_PER_CORE // SC_ROWS       # 8
CFD = SC_ROWS // 4                    # psum free dim per batch group (1024)


def _schedule():
    """Superchunk (offset, rows) list."""
    return [(j * SC_ROWS, SC_ROWS) for j in range(N_SC)]
MAGIC = 12582912.0                    # 1.5 * 2**23
IR_DROP = 0.99
LOGNORMAL_SIGMA = 0.12

_CACHE = {}


def _noise_factors(w_shapes):
    """Reproduce the reference's per-weight lognormal conductance noise."""
    import jax

    cpu = jax.devices("cpu")[0]
    with jax.default_device(cpu):
        import jax.numpy as jnp

        ks = jax.random.split(jax.random.key(42), 8)
        gs = []
        for i, shape in enumerate(w_shapes):
            g = jnp.exp(
                jax.random.normal(ks[2 * i], shape, dtype=jnp.float32)
                * LOGNORMAL_SIGMA
            )
            gs.append(np.asarray(g, dtype=np.float32))
    return gs


def _build_nc(n_sc):
    from contextlib import ExitStack

    import concourse.tile as tile
    from concourse import bacc, mybir

    dt = mybir.dt
    AF = mybir.ActivationFunctionType
    AL = mybir.AluOpType

    nc = bacc.Bacc("TRN2", target_bir_lowering=False, debug=False)
    scs = _schedule() if n_sc == N_SC else [
        (j * SC_ROWS, SC_ROWS) for j in range(n_sc)
    ]
    rows = sum(r for _, r in scs)
    xT = nc.dram_tensor("xt", [128, rows], dt.float16, kind="ExternalInput")
    wA1 = nc.dram_tensor("wa1", [128, 64], dt.float16, kind="ExternalInput")
    wF = nc.dram_tensor("wf", [128, 192], dt.float16, kind="ExternalInput")
    out = nc.dram_tensor("out", [128, rows // 4], dt.float16, kind="ExternalOutput")

    with tile.TileContext(nc) as tc, ExitStack() as ctx:
        wpool = ctx.enter_context(tc.tile_pool(name="w", bufs=1))
        xpool = ctx.enter_context(tc.tile_pool(name="x", bufs=3))
        spool = ctx.enter_context(tc.tile_pool(name="s", bufs=4))
        opool = ctx.enter_context(tc.tile_pool(name="o", bufs=3))
        ppools = [
            ctx.enter_context(tc.tile_pool(name=f"p{i}", bufs=1, space="PSUM"))
            for i in range(1, 5)
        ]

        w1s = wpool.tile([128, 64], dt.float16, tag="w1s")
        nc.sync.dma_start(w1s[:], wA1[:, :])
        wfs = wpool.tile([128, 192], dt.float16, tag="wfs")
        nc.sync.dma_start(wfs[:], wF[:, :])

        def split_mm(ps, hi_cols, lo_cols, rhs, k, cfd):
            """fp16 hi+lo weight matmuls, wave-ordered (all hi, then all lo)
            so the four diagonal tile-groups can run concurrently."""
            for cols, st, sp in ((hi_cols, True, False), (lo_cols, False, True)):
                for ch in range(0, cfd, 512):
                    for g in range(4):
                        nc.tensor.matmul(
                            ps[32 * g : 32 * g + 32, ch : ch + 512],
                            wfs[32 * g : 32 * g + k, cols[0] : cols[1]],
                            rhs[32 * g : 32 * g + k, ch : ch + 512],
                            start=st,
                            stop=sp,
                            skip_group_check=True,
                            tile_position=(32 * g, 32 * g),
                        )

        # Software-pipelined emission: at iteration i, stage-l work runs on
        # superchunk i-l, so every TensorE instruction's inputs were produced
        # in an earlier iteration and PE never stalls on the current
        # superchunk's elementwise chain (also keeps PE dense -> HAM warm).
        s1s, s2s, s3s = {}, {}, {}
        xts = {}
        n = len(scs)

        def load_x(j):
            off, rws = scs[j]
            xt = xpool.tile([128, rws], dt.float16, tag="xt")
            nc.sync.dma_start(xt[:], xT[:, off : off + rws])
            xts[j] = xt

        # HAM warmup: ~3.4us of dependency-free PE work overlapping the first
        # x DMA, so the real matmuls start at the un-throttled clock.
        wu = spool.tile([128, 512], dt.float16, tag="wu")
        nc.vector.memset(wu[:], 1.0)
        pw = ppools[0].tile([128, 512], dt.float32, tag="ps1")
        for r in range(8):
            nc.tensor.matmul(
                pw[0:32, :],
                wu[0:128, 0:32],
                wu[0:128, :],
                start=(r == 0),
                stop=(r == 7),
                skip_group_check=True,
                tile_position=(0, 0),
            )

        load_x(0)
        for i in range(n + 3):
            if i < n:
                _, rws = scs[i]
                cfd = rws // 4
                xt = xts.pop(i)

                # layer 1: xt holds xh (partitions 0-63) and xl (64-127) for
                # the same batch. Wave A: K=128 with [A1h;A1h] stacked computes
                # (xh+xl)@A1h = x@A1h; wave B: [A1l;A1l] adds x@A1l.
                ps1 = ppools[0].tile([128, cfd], dt.float32, tag="ps1")
                for wc, st, sp in (((0, 32), True, False), ((32, 64), False, True)):
                    for ch in range(0, cfd, 512):
                        for g in range(4):
                            nc.tensor.matmul(
                                ps1[32 * g : 32 * g + 32, ch : ch + 512],
                                w1s[0:128, wc[0] : wc[1]],
                                xt[0:128, g * cfd + ch : g * cfd + ch + 512],
                                start=st,
                                stop=sp,
                                skip_group_check=True,
                                tile_position=(0, 32 * g),
                            )
                # clamp+round in one op: fp32->int16 convert rounds-to-nearest-even
                t1 = spool.tile([128, cfd], dt.int16, tag="t1")
                nc.vector.tensor_scalar(
                    t1[:], ps1[:], 0.0, 128.0, op0=AL.max, op1=AL.min
                )
                s1 = spool.tile([128, cfd], dt.float16, tag="s1")
                nc.vector.tensor_copy(s1[:], t1[:])
                s1s[i] = (s1, cfd)

            if 0 <= i - 1 < n:
                s1, cfd = s1s.pop(i - 1)
                ps2 = ppools[1].tile([128, cfd], dt.float32, tag="ps2")
                split_mm(ps2, (0, 32), (32, 64), s1, 32, cfd)
                # relu+round in one ACT op (int16 out converts with RNE);
                # then clamp-top+cast in one DVE op
                u2 = spool.tile([128, cfd], dt.int16, tag="u2")
                nc.scalar.activation(u2[:], ps2[:], AF.Relu)
                s2 = spool.tile([128, cfd], dt.float16, tag="s2")
                nc.vector.tensor_scalar(s2[:], u2[:], 128.0, None, op0=AL.min)
                s2s[i - 1] = (s2, cfd)

            if 0 <= i - 2 < n:
                s2, cfd = s2s.pop(i - 2)
                ps3 = ppools[2].tile([128, cfd], dt.float32, tag="ps3")
                split_mm(ps3, (64, 96), (96, 128), s2, 24, cfd)
                u3 = spool.tile([128, cfd], dt.int16, tag="u3")
                nc.scalar.activation(u3[:], ps3[:], AF.Relu)
                s3 = spool.tile([128, cfd], dt.float16, tag="s3")
                nc.vector.tensor_scalar(s3[:], u3[:], 128.0, None, op0=AL.min)
                s3s[i - 2] = (s3, cfd)

            if 0 <= i - 3 < n:
                off, rws = scs[i - 3]
                s3, cfd = s3s.pop(i - 3)
                ps4 = ppools[3].tile([128, cfd], dt.float32, tag="ps4")
                for ch in range(0, cfd, 512):
                    for g in range(4):
                        nc.tensor.matmul(
                            ps4[32 * g : 32 * g + 32, ch : ch + 512],
                            wfs[32 * g : 32 * g + 32, 128:160],
                            s3[32 * g : 32 * g + 32, ch : ch + 512],
                            start=True,
                            stop=True,
                            skip_group_check=True,
                            tile_position=(32 * g, 32 * g),
                        )
                q4 = spool.tile([128, cfd], dt.int16, tag="q4")
                nc.scalar.activation(q4[:], ps4[:], AF.Copy)
                o = opool.tile([128, cfd], dt.float16, tag="o")
                nc.scalar.activation(o[:], q4[:], AF.Tanh, scale=1.0 / 128.0)
                nc.sync.dma_start(out[:, off // 4 : off // 4 + cfd], o[:])

            if 0 <= i + 1 < n:
                load_x(i + 1)

    return nc


def _prep_weights(w1, w2, w3, w4):
    gs = _noise_factors([w.shape for w in (w1, w2, w3, w4)])
    A1 = (128.0 * IR_DROP * (w1.astype(np.float32) * gs[0])).T.astype(np.float32)
    A2 = (IR_DROP * (w2.astype(np.float32) * gs[1])).T.astype(np.float32)  # [32,24]
    A3 = (IR_DROP * (w3.astype(np.float32) * gs[2])).T.astype(np.float32)  # [24,16]
    A4 = (IR_DROP * (w4.astype(np.float32) * gs[3])).T.astype(np.float32)  # [16, 8]

    def split16(a):
        hi = a.astype(np.float16)
        lo = (a - hi.astype(np.float32)).astype(np.float16)
        return hi, lo

    A1h, A1l = split16(A1)
    w1pack = np.zeros((128, 64), dtype=np.float16)
    w1pack[:, 0:32] = np.vstack([A1h, A1h])
    w1pack[:, 32:64] = np.vstack([A1l, A1l])

    A2h, A2l = split16(A2)
    A3h, A3l = split16(A3)
    A4h, A4l = split16(A4)
    wfpack = np.zeros((128, 192), dtype=np.float16)
    for g in range(4):
        wfpack[32 * g : 32 * g + 32, 0:24] = A2h
        wfpack[32 * g : 32 * g + 32, 32:56] = A2l
        # layer 3 emits h3 twice (cols 0:16 and 16:32 of its M=32 output)...
        wfpack[32 * g : 32 * g + 24, 64:80] = A3h
        wfpack[32 * g : 32 * g + 24, 80:96] = A3h
        wfpack[32 * g : 32 * g + 24, 96:112] = A3l
        wfpack[32 * g : 32 * g + 24, 112:128] = A3l
        # ...so layer 4 is ONE K=32 matmul: [h3;h3] @ [A4h;A4l] = h3 @ A4
        wfpack[32 * g : 32 * g + 16, 128:136] = A4h
        wfpack[32 * g + 16 : 32 * g + 32, 128:136] = A4l
    return w1pack, wfpack


def _pack_x_shard(xs):
    """[rows, 64] fp32 -> [128, rows] fp16: partitions 0-63 hold xh (fp16 hi),
    partitions 64-127 hold xl (fp16 residual), batch in natural order."""
    xs = xs.astype(np.float32)
    xh = xs.astype(np.float16)
    xl = (xs - xh.astype(np.float32)).astype(np.float16)
    return np.ascontiguousarray(np.concatenate([xh.T, xl.T], axis=0))


def _decode_out(dev_out, rows):
    """[128, rows//4] fp16 -> [rows, 8] fp32; per superchunk (off, rws) the
    block cols off//4.. holds feature r of batch off + g*cfd + c at row
    32g+r, col off//4 + c."""
    res = np.empty((rows, ACTION_DIM), dtype=np.float32)
    f32 = dev_out.astype(np.float32)
    for off, rws in _schedule():
        cfd = rws // 4
        blk = f32[:, off // 4 : off // 4 + cfd]
        for g in range(4):
            base = off + g * cfd
            res[base : base + cfd, :] = blk[32 * g : 32 * g + 8, :].T
    return res


def run(inputs, trace=False):
    x = np.asarray(inputs["x"], dtype=np.float32)
    assert x.shape == (B, STATE_DIM)
    w1pack, wfpack = _prep_weights(
        np.asarray(inputs["w1"]), np.asarray(inputs["w2"]),
        np.asarray(inputs["w3"]), np.asarray(inputs["w4"]),
    )

    if "nc" not in _CACHE:
        nc = _build_nc(N_SC)
        if not nc.is_finalized():
            nc.finalize()
        _CACHE["nc"] = nc
    nc = _CACHE["nc"]

    in_maps = []
    for c in range(N_CORES):
        shard = x[c * ROWS_PER_CORE : (c + 1) * ROWS_PER_CORE]
        in_maps.append(
            {"xt": _pack_x_shard(shard), "wa1": w1pack, "wf": wfpack}
        )

    from concourse.bass_utils import run_bass_kernel_spmd

    res = run_bass_kernel_spmd(
        nc, in_maps, core_ids=list(range(N_CORES)), trace=trace
    )

    full = np.empty((B, ACTION_DIM), dtype=np.float32)
    for c in range(N_CORES):
        full[c * ROWS_PER_CORE : (c + 1) * ROWS_PER_CORE] = _decode_out(
            res.results[c]["out"], ROWS_PER_CORE
        )
    return full, res.exec_time_ns


def kernel(**inputs):
    out, _ = run(inputs)
    return out


# revision 77
# speedup vs baseline: 1.1932x; 1.1471x over previous
"""Trainium2 Bass kernel for nn_Actor_47278999995139 (dense_mlp, memory-bound).

Reference computation (per row of x, B=262144 rows):
    4-layer MLP 64->32->24->16->8; each layer:
        y   = (h_in * 0.99) @ (w * g).T        (g = lognormal weight noise)
        q   = clip(round(y * 128) / 128, -1, 1)   (+ tiny thermal noise, omitted)
        h   = relu(q)   (layers 1-3);  out = tanh(q)  (layer 4)

Strategy: pure data parallel over 8 NeuronCores (32768 rows/core).
  * All scale factors fold into host-precomputed weight matrices, so the
    device works in the h*128 integer domain:
        A1 = (128*0.99*w1*g1).T      psum1 = x @ A1           = y1*128
        Ak = (0.99*wk*gk).T          psumk = h_{k-1}s @ Ak    = yk*128
    and h_s = clip(round(y*128), 0, 128) feeds the next matmul directly.
  * Host pre-transposes x so the device sees [features, batch] tiles
    (contraction on partitions, no on-device transposes). x is sent as an
    exact fp16 hi/lo split (same 4 B/elem as fp32): hi on partitions 0-63,
    lo on 64-127, so layer 1 is two K=128 fp16 matmuls with [A1;A1]-stacked
    split weights. Layers 2-4 use fp16 hi+lo split weights (exact to 2^-22);
    h values are integers 0..128, exact in fp16. Layer 3 emits h3 twice
    (A3's columns duplicated into its zero-pad half) so layer 4's hi+lo
    pair collapses into ONE K=32 matmul with [A4h;A4l] stacked.
  * Batch is packed 4096-rows-per-superchunk across all 128 partitions as
    4 diagonal (row,col) tile-groups of 32 - the four groups' matmuls run
    concurrently in the PE array (wave-ordered: all hi then all lo), and
    DVE/ACT lanes stay full for the elementwise work.
  * clip+round exploits that the hardware fp32->int16 output convert
    rounds half-to-even, exactly matching jnp.round: layer 1 is one DVE
    tensor_scalar (max,min)->int16 + cast; layers 2/3 are one ACT
    Relu->int16 + one DVE min->fp16; layer 4 is one ACT Copy->int16.
    (The python CoreSim diverges here - it truncates - so numerical
    correctness is validated against the reference on hardware.)
  * The emission is software-pipelined 4 deep (stage l of superchunk i-l at
    iteration i) so TensorE never waits on the current superchunk's
    elementwise chain and stays HAM-warm.
  * Layer-4's +-1 clip is dropped: P(|y4|>1) is ~0 (y4 std ~0.16) and tanh
    saturates; output is written as fp16 (integers + tanh, exact enough).
  * Per-weight lognormal noise is reproduced exactly on host with jax CPU;
    per-element thermal noise (sigma=8e-7) is omitted (~5.4e-3 relative
    error, dominated by rounding-bin flips at quantization boundaries -
    reproducing it exactly would require streaming 84 MB of host-generated
    noise, doubling the memory traffic).
"""

import os
import sys

import numpy as np

sys.path.insert(0, "/opt/trn_rl_repo")

B = 262144
STATE_DIM = 64
ACTION_DIM = 8
N_CORES = 8
ROWS_PER_CORE = B // N_CORES          # 32768
HALF = ROWS_PER_CORE // 2             # 16384 (batch cols per partition-half)
SC_ROWS = 4096                        # rows per (full) superchunk
N_SC = ROWS_PER_CORE // SC_ROWS       # 8
CFD = SC_ROWS // 4                    # psum free dim per batch group (1024)


def _schedule():
    """Superchunk (offset, rows) list."""
    return [(j * SC_ROWS, SC_ROWS) for j in range(N_SC)]
MAGIC = 12582912.0                    # 1.5 * 2**23
IR_DROP = 0.99
LOGNORMAL_SIGMA = 0.12

_CACHE = {}


def _noise_factors(w_shapes):
    """Reproduce the reference's per-weight lognormal conductance noise."""
    import jax

    cpu = jax.devices("cpu")[0]
    with jax.default_device(cpu):
        import jax.numpy as jnp

        ks = jax.random.split(jax.random.key(42), 8)
        gs = []
        for i, shape in enumerate(w_shapes):
            g = jnp.exp(
                jax.random.normal(ks[2 * i], shape, dtype=jnp.float32)
                * LOGNORMAL_SIGMA
            )
            gs.append(np.asarray(g, dtype=np.float32))
    return gs


def _build_nc(n_sc):
    from contextlib import ExitStack

    import concourse.tile as tile
    from concourse import bacc, mybir

    dt = mybir.dt
    AF = mybir.ActivationFunctionType
    AL = mybir.AluOpType

    nc = bacc.Bacc("TRN2", target_bir_lowering=False, debug=False)
    scs = _schedule() if n_sc == N_SC else [
        (j * SC_ROWS, SC_ROWS) for j in range(n_sc)
    ]
    rows = sum(r for _, r in scs)
    xT = nc.dram_tensor("xt", [128, rows], dt.float16, kind="ExternalInput")
    wA1 = nc.dram_tensor("wa1", [128, 64], dt.float16, kind="ExternalInput")
    wF = nc.dram_tensor("wf", [128, 192], dt.float16, kind="ExternalInput")
    out = nc.dram_tensor("out", [128, rows // 4], dt.float16, kind="ExternalOutput")

    with tile.TileContext(nc) as tc, ExitStack() as ctx:
        wpool = ctx.enter_context(tc.tile_pool(name="w", bufs=1))
        xpool = ctx.enter_context(tc.tile_pool(name="x", bufs=3))
        spool = ctx.enter_context(tc.tile_pool(name="s", bufs=4))
        opool = ctx.enter_context(tc.tile_pool(name="o", bufs=3))
        ppools = [
            ctx.enter_context(tc.tile_pool(name=f"p{i}", bufs=1, space="PSUM"))
            for i in range(1, 5)
        ]

        w1s = wpool.tile([128, 64], dt.float16, tag="w1s")
        nc.sync.dma_start(w1s[:], wA1[:, :])
        wfs = wpool.tile([128, 192], dt.float16, tag="wfs")
        nc.sync.dma_start(wfs[:], wF[:, :])

        def split_mm(ps, hi_cols, lo_cols, rhs, k, cfd):
            """fp16 hi+lo weight matmuls, wave-ordered (all hi, then all lo)
            so the four diagonal tile-groups can run concurrently."""
            for cols, st, sp in ((hi_cols, True, False), (lo_cols, False, True)):
                for ch in range(0, cfd, 512):
                    for g in range(4):
                        nc.tensor.matmul(
                            ps[32 * g : 32 * g + 32, ch : ch + 512],
                            wfs[32 * g : 32 * g + k, cols[0] : cols[1]],
                            rhs[32 * g : 32 * g + k, ch : ch + 512],
                            start=st,
                            stop=sp,
                            skip_group_check=True,
                            tile_position=(32 * g, 32 * g),
                        )

        # Software-pipelined emission: at iteration i, stage-l work runs on
        # superchunk i-l, so every TensorE instruction's inputs were produced
        # in an earlier iteration and PE never stalls on the current
        # superchunk's elementwise chain (also keeps PE dense -> HAM warm).
        s1s, s2s, s3s = {}, {}, {}
        xts = {}
        n = len(scs)

        def load_x(j):
            off, rws = scs[j]
            xt = xpool.tile([128, rws], dt.float16, tag="xt")
            nc.sync.dma_start(xt[:], xT[:, off : off + rws])
            xts[j] = xt

        # HAM warmup: ~3.4us of dependency-free PE work overlapping the first
        # x DMA, so the real matmuls start at the un-throttled clock.
        wu = spool.tile([128, 512], dt.float16, tag="wu")
        nc.vector.memset(wu[:], 1.0)
        pw = ppools[0].tile([128, 512], dt.float32, tag="ps1")
        for r in range(8):
            nc.tensor.matmul(
                pw[0:32, :],
                wu[0:128, 0:32],
                wu[0:128, :],
                start=(r == 0),
                stop=(r == 7),
                skip_group_check=True,
                tile_position=(0, 0),
            )

        load_x(0)
        for i in range(n + 3):
            if i < n:
                _, rws = scs[i]
                cfd = rws // 4
                xt = xts.pop(i)

                # layer 1: xt holds xh (partitions 0-63) and xl (64-127) for
                # the same batch. Wave A: K=128 with [A1h;A1h] stacked computes
                # (xh+xl)@A1h = x@A1h; wave B: [A1l;A1l] adds x@A1l.
                ps1 = ppools[0].tile([128, cfd], dt.float32, tag="ps1")
                for wc, st, sp in (((0, 32), True, False), ((32, 64), False, True)):
                    for ch in range(0, cfd, 512):
                        for g in range(4):
                            nc.tensor.matmul(
                                ps1[32 * g : 32 * g + 32, ch : ch + 512],
                                w1s[0:128, wc[0] : wc[1]],
                                xt[0:128, g * cfd + ch : g * cfd + ch + 512],
                                start=st,
                                stop=sp,
                                skip_group_check=True,
                                tile_position=(0, 32 * g),
                            )
                # clamp+round in one op: fp32->int16 convert rounds-to-nearest-even
                t1 = spool.tile([128, cfd], dt.int16, tag="t1")
                nc.vector.tensor_scalar(
                    t1[:], ps1[:], 0.0, 128.0, op0=AL.max, op1=AL.min
                )
                s1 = spool.tile([128, cfd], dt.float16, tag="s1")
                nc.vector.tensor_copy(s1[:], t1[:])
                s1s[i] = (s1, cfd)

            if 0 <= i - 1 < n:
                s1, cfd = s1s.pop(i - 1)
                ps2 = ppools[1].tile([128, cfd], dt.float32, tag="ps2")
                split_mm(ps2, (0, 32), (32, 64), s1, 32, cfd)
                # relu+round in one ACT op (int16 out converts with RNE);
                # then clamp-top+cast in one DVE op
                u2 = spool.tile([128, cfd], dt.int16, tag="u2")
                nc.scalar.activation(u2[:], ps2[:], AF.Relu)
                s2 = spool.tile([128, cfd], dt.float16, tag="s2")
                nc.vector.tensor_scalar(s2[:], u2[:], 128.0, None, op0=AL.min)
                s2s[i - 1] = (s2, cfd)

            if 0 <= i - 2 < n:
                s2, cfd = s2s.pop(i - 2)
                ps3 = ppools[2].tile([128, cfd], dt.float32, tag="ps3")
                split_mm(ps3, (64, 96), (96, 128), s2, 24, cfd)
                u3 = spool.tile([128, cfd], dt.int16, tag="u3")
                nc.scalar.activation(u3[:], ps3[:], AF.Relu)
                s3 = spool.tile([128, cfd], dt.float16, tag="s3")
                nc.vector.tensor_scalar(s3[:], u3[:], 128.0, None, op0=AL.min)
                s3s[i - 2] = (s3, cfd)

            if 0 <= i - 3 < n:
                off, rws = scs[i - 3]
                s3, cfd = s3s.pop(i - 3)
                ps4 = ppools[3].tile([128, cfd], dt.float32, tag="ps4")
                for ch in range(0, cfd, 512):
                    for g in range(4):
                        nc.tensor.matmul(
                            ps4[32 * g : 32 * g + 32, ch : ch + 512],
                            wfs[32 * g : 32 * g + 32, 128:160],
                            s3[32 * g : 32 * g + 32, ch : ch + 512],
                            start=True,
                            stop=True,
                            skip_group_check=True,
                            tile_position=(32 * g, 32 * g),
                        )
                q4 = spool.tile([128, cfd], dt.int16, tag="q4")
                nc.scalar.activation(q4[:], ps4[:], AF.Copy)
                o = opool.tile([128, cfd], dt.float16, tag="o")
                nc.scalar.activation(o[:], q4[:], AF.Tanh, scale=1.0 / 128.0)
                nc.sync.dma_start(out[:, off // 4 : off // 4 + cfd], o[:])

            if 0 <= i + 1 < n:
                load_x(i + 1)

    return nc


def _prep_weights(w1, w2, w3, w4):
    gs = _noise_factors([w.shape for w in (w1, w2, w3, w4)])
    A1 = (128.0 * IR_DROP * (w1.astype(np.float32) * gs[0])).T.astype(np.float32)
    A2 = (IR_DROP * (w2.astype(np.float32) * gs[1])).T.astype(np.float32)  # [32,24]
    A3 = (IR_DROP * (w3.astype(np.float32) * gs[2])).T.astype(np.float32)  # [24,16]
    A4 = (IR_DROP * (w4.astype(np.float32) * gs[3])).T.astype(np.float32)  # [16, 8]

    def split16(a):
        hi = a.astype(np.float16)
        lo = (a - hi.astype(np.float32)).astype(np.float16)
        return hi, lo

    A1h, A1l = split16(A1)
    w1pack = np.zeros((128, 64), dtype=np.float16)
    w1pack[:, 0:32] = np.vstack([A1h, A1h])
    w1pack[:, 32:64] = np.vstack([A1l, A1l])

    A2h, A2l = split16(A2)
    A3h, A3l = split16(A3)
    A4h, A4l = split16(A4)
    wfpack = np.zeros((128, 192), dtype=np.float16)
    for g in range(4):
        wfpack[32 * g : 32 * g + 32, 0:24] = A2h
        wfpack[32 * g : 32 * g + 32, 32:56] = A2l
        # layer 3 emits h3 twice (cols 0:16 and 16:32 of its M=32 output)...
        wfpack[32 * g : 32 * g + 24, 64:80] = A3h
        wfpack[32 * g : 32 * g + 24, 80:96] = A3h
        wfpack[32 * g : 32 * g + 24, 96:112] = A3l
        wfpack[32 * g : 32 * g + 24, 112:128] = A3l
        # ...so layer 4 is ONE K=32 matmul: [h3;h3] @ [A4h;A4l] = h3 @ A4
        wfpack[32 * g : 32 * g + 16, 128:136] = A4h
        wfpack[32 * g + 16 : 32 * g + 32, 128:136] = A4l
    return w1pack, wfpack


def _pack_x_shard(xs):
    """[rows, 64] fp32 -> [128, rows] fp16: partitions 0-63 hold xh (fp16 hi),
    partitions 64-127 hold xl (fp16 residual), batch in natural order."""
    xs = xs.astype(np.float32)
    xh = xs.astype(np.float16)
    xl = (xs - xh.astype(np.float32)).astype(np.float16)
    return np.ascontiguousarray(np.concatenate([xh.T, xl.T], axis=0))


def _decode_out(dev_out, rows):
    """[128, rows//4] fp16 -> [rows, 8] fp32; per superchunk (off, rws) the
    block cols off//4.. holds feature r of batch off + g*cfd + c at row
    32g+r, col off//4 + c."""
    res = np.empty((rows, ACTION_DIM), dtype=np.float32)
    f32 = dev_out.astype(np.float32)
    for off, rws in _schedule():
        cfd = rws // 4
        blk = f32[:, off // 4 : off // 4 + cfd]
        for g in range(4):
            base = off + g * cfd
            res[base : base + cfd, :] = blk[32 * g : 32 * g + 8, :].T
    return res


def run(inputs, trace=False):
    x = np.asarray(inputs["x"], dtype=np.float32)
    assert x.shape == (B, STATE_DIM)
    w1pack, wfpack = _prep_weights(
        np.asarray(inputs["w1"]), np.asarray(inputs["w2"]),
        np.asarray(inputs["w3"]), np.asarray(inputs["w4"]),
    )

    if "nc" not in _CACHE:
        nc = _build_nc(N_SC)
        if not nc.is_finalized():
            nc.finalize()
        _CACHE["nc"] = nc
    nc = _CACHE["nc"]

    in_maps = []
    for c in range(N_CORES):
        shard = x[c * ROWS_PER_CORE : (c + 1) * ROWS_PER_CORE]
        in_maps.append(
            {"xt": _pack_x_shard(shard), "wa1": w1pack, "wf": wfpack}
        )

    from concourse.bass_utils import run_bass_kernel_spmd

    res = run_bass_kernel_spmd(
        nc, in_maps, core_ids=list(range(N_CORES)), trace=trace
    )

    full = np.empty((B, ACTION_DIM), dtype=np.float32)
    for c in range(N_CORES):
        full[c * ROWS_PER_CORE : (c + 1) * ROWS_PER_CORE] = _decode_out(
            res.results[c]["out"], ROWS_PER_CORE
        )
    return full, res.exec_time_ns


def kernel(**inputs):
    out, _ = run(inputs)
    return out
